# revision 5
# baseline (speedup 1.0000x reference)
"""GATv2 block kernel for 8 Trainium2 NeuronCores (Bass/Tile).

Strategy (graph/data parallel over destination nodes):
  - Host sorts edges by destination, shards destination nodes across the
    8 cores (6250 nodes each, padded to 6272 = 49 tiles of 128).
  - Per destination-node tile, edges are padded to multiples of 128
    ("chunks"); chunk counts per tile are maxed across cores so one SPMD
    program serves all 8 cores.
  - Host supplies x[src] pre-gathered AND transposed (x_srcT) so the
    device computes per-edge xl[src] = w_l @ x_src via matmuls with a
    constant stationary operand (no indirect DMA gathers).
  - Segment softmax + scatter-add are matmuls against indicator matrices
    I[e,n] = (dst_local[e] == n) built on-device with is_equal.
  - exp/leaky_relu live in one ACT table set; silu + sqrt run in a tail
    phase (one table switch each).
"""

import numpy as np

P = 128
HEADS = 4
HEAD_DIM = 32
OUT_DIM = 128
IN_DIM = 128
EDGE_DIM = 10
NEG_SLOPE = 0.2
LN_EPS = 1e-5
N_CORES = 8
SUPER = 4  # chunks per superchunk (free dim 512)

_CACHE = {}


def _build_program(C_list, trivial_affine):
    import concourse.bacc as bacc
    import concourse.bass as bass
    import concourse.tile as tile
    from concourse import mybir

    f32 = mybir.dt.float32
    AT = mybir.ActivationFunctionType
    OP = mybir.AluOpType

    NT = len(C_list)                       # 49 node tiles per core
    TOTAL_CHUNKS = sum(C_list)
    NPC_PAD = NT * P                       # 6272
    EW = TOTAL_CHUNKS * P                  # padded edges per core

    nc = bacc.Bacc('TRN2', target_bir_lowering=False, debug=False,
                   enable_asserts=True, num_devices=N_CORES)

    # ---- external inputs ----
    x_srcT = nc.dram_tensor('x_srcT', [P, EW], f32, kind='ExternalInput')
    attrT = nc.dram_tensor('attrT', [EDGE_DIM, EW], f32, kind='ExternalInput')
    dstrow = nc.dram_tensor('dstrow', [1, EW], f32, kind='ExternalInput')
    dstloc = nc.dram_tensor('dstloc', [P, TOTAL_CHUNKS], f32, kind='ExternalInput')
    x_ownT = nc.dram_tensor('x_ownT', [P, NPC_PAD], f32, kind='ExternalInput')
    x_own = nc.dram_tensor('x_own', [NPC_PAD, P], f32, kind='ExternalInput')
    w_lT = nc.dram_tensor('w_lT', [P, P], f32, kind='ExternalInput')
    w_rT = nc.dram_tensor('w_rT', [P, P], f32, kind='ExternalInput')
    w_eT = nc.dram_tensor('w_eT', [EDGE_DIM, P], f32, kind='ExternalInput')
    att_exp = nc.dram_tensor('att_exp', [P, HEADS], f32, kind='ExternalInput')
    iota_row = nc.dram_tensor('iota_row', [P, P], f32, kind='ExternalInput')
    iota_col = nc.dram_tensor('iota_col', [P, 1], f32, kind='ExternalInput')
    ones_row = nc.dram_tensor('ones_row', [1, P], f32, kind='ExternalInput')
    id4 = nc.dram_tensor('id4', [HEADS, HEADS], f32, kind='ExternalInput')
    bias_lr = nc.dram_tensor('bias_lr', [P, 1], f32, kind='ExternalInput')
    aff = None
    if not trivial_affine:
        # rows: b_l bcast, conv_bias bcast, gamma bcast, beta bcast
        aff = nc.dram_tensor('aff', [P, 4 * P], f32, kind='ExternalInput')

    out_d = nc.dram_tensor('out', [NPC_PAD, P], f32, kind='ExternalOutput')

    with tile.TileContext(nc) as tc:
        with tc.tile_pool(name='const', bufs=1) as cp:
            c_wlT = cp.tile([P, P], f32)
            nc.sync.dma_start(c_wlT[:], w_lT[:])
            c_wrT = cp.tile([P, P], f32)
            nc.sync.dma_start(c_wrT[:], w_rT[:])
            c_weT = cp.tile([EDGE_DIM, P], f32)
            nc.sync.dma_start(c_weT[:], w_eT[:])
            c_att = cp.tile([P, HEADS], f32)
            nc.sync.dma_start(c_att[:], att_exp[:])
            c_iota = cp.tile([P, P], f32)
            nc.sync.dma_start(c_iota[:], iota_row[:])
            c_iotac = cp.tile([P, 1], f32)
            nc.sync.dma_start(c_iotac[:], iota_col[:])
            c_ones = cp.tile([1, P], f32)
            nc.sync.dma_start(c_ones[:], ones_row[:])
            c_id4 = cp.tile([HEADS, HEADS], f32)
            nc.sync.dma_start(c_id4[:], id4[:])
            c_blr = cp.tile([P, 1], f32)
            nc.sync.dma_start(c_blr[:], bias_lr[:])
            c_xownT = cp.tile([P, NPC_PAD], f32)
            nc.sync.dma_start(c_xownT[:], x_ownT[:])
            c_aff = None
            if aff is not None:
                c_aff = cp.tile([P, 4 * P], f32)
                nc.sync.dma_start(c_aff[:], aff[:])

            with tc.tile_pool(name='persist', bufs=1) as pp:
                xr_sb = pp.tile([P, NT * P], f32)      # xr per node tile
                ubuf = pp.tile([P, NT * 132], f32)     # unnorm(128)+denom(4)
                hbuf = pp.tile([P, NT * P], f32)       # post-residual h
                stats = pp.tile([P, NT * 2], f32)      # mean, var interleaved

                # ---------- phase 1: xr for own nodes ----------
                with tc.tile_pool(name='p1psum', bufs=2, space='PSUM') as p1p:
                    for t in range(NT):
                        ps = p1p.tile([P, P], f32)
                        nc.tensor.matmul(ps[:], lhsT=c_xownT[:, t * P:(t + 1) * P],
                                         rhs=c_wrT[:], start=True, stop=True)
                        nc.vector.tensor_copy(xr_sb[:, t * P:(t + 1) * P], ps[:])

                # ---------- phase 2: edge pipeline ----------
                with tc.tile_pool(name='eload', bufs=3) as lp, \
                     tc.tile_pool(name='ework', bufs=2) as wp, \
                     tc.tile_pool(name='psA', bufs=2, space='PSUM') as psA, \
                     tc.tile_pool(name='psB', bufs=1, space='PSUM') as psB, \
                     tc.tile_pool(name='psC', bufs=1, space='PSUM') as psC, \
                     tc.tile_pool(name='psD', bufs=2, space='PSUM') as psD, \
                     tc.tile_pool(name='psO', bufs=1, space='PSUM') as psO:
                    chunk_base = 0
                    for t in range(NT):
                        Ct = C_list[t]
                        dl_t = lp.tile([P, Ct], f32, tag='dl')
                        nc.sync.dma_start(
                            dl_t[:], dstloc[:, chunk_base:chunk_base + Ct])
                        ps_out = psO.tile([P, 132], f32, tag='out')
                        xr_t = xr_sb[:, t * P:(t + 1) * P]
                        n_super = (Ct + SUPER - 1) // SUPER
                        for s in range(n_super):
                            nch = min(SUPER, Ct - s * SUPER)
                            W = nch * P
                            e0 = (chunk_base + s * SUPER) * P
                            xsT = lp.tile([P, SUPER * P], f32, tag='xsT')
                            nc.sync.dma_start(xsT[:, :W], x_srcT[:, e0:e0 + W])
                            atr = lp.tile([EDGE_DIM, SUPER * P], f32, tag='atr')
                            nc.sync.dma_start(atr[:, :W], attrT[:, e0:e0 + W])
                            dr = lp.tile([1, SUPER * P], f32, tag='dr')
                            nc.sync.dma_start(dr[:, :W], dstrow[:, e0:e0 + W])

                            # s^T = xj^T + ea^T + xr[dst]^T   (feature-major)
                            ps_sT = psA.tile([P, SUPER * P], f32, tag='sT')
                            nc.tensor.matmul(ps_sT[:, :W], lhsT=c_wlT[:],
                                             rhs=xsT[:, :W], start=True, stop=False)
                            nc.tensor.matmul(ps_sT[:, :W], lhsT=c_weT[:],
                                             rhs=atr[:, :W], start=False, stop=False)
                            ps_b = psB.tile([P, SUPER * P], f32, tag='bc')
                            nc.tensor.matmul(ps_b[:, :W], lhsT=c_ones[:],
                                             rhs=dr[:, :W], start=True, stop=True)
                            IT = wp.tile([P, SUPER * P], f32, tag='IT')
                            nc.vector.tensor_scalar(
                                out=IT[:, :W], in0=ps_b[:, :W],
                                scalar1=c_iotac[:], scalar2=None, op0=OP.is_equal)
                            nc.tensor.matmul(ps_sT[:, :W], lhsT=xr_t,
                                             rhs=IT[:, :W], start=False, stop=True)

                            # m = lrelu(s + (b_l+b_r))  (bias per feature row)
                            m = wp.tile([P, SUPER * P], f32, tag='m')
                            nc.scalar.activation(m[:, :W], ps_sT[:, :W], AT.Prelu,
                                                 bias=c_blr[:], alpha=NEG_SLOPE)

                            # logits = att_exp^T @ m  -> [4, W]
                            ps_lg = psC.tile([HEADS, SUPER * P], f32, tag='lg')
                            nc.tensor.matmul(ps_lg[:, :W], lhsT=c_att[:],
                                             rhs=m[:, :W], start=True, stop=True)
                            exT = wp.tile([HEADS, SUPER * P], f32, tag='exT')
                            nc.scalar.activation(exT[:, :W], ps_lg[:, :W], AT.Exp)

                            # transpose ex to edge-major [128, nch*4]
                            ps_ex = psC.tile([P, SUPER * HEADS], f32, tag='ex')
                            for j in range(nch):
                                nc.tensor.matmul(
                                    ps_ex[:, j * HEADS:(j + 1) * HEADS],
                                    lhsT=exT[:, j * P:(j + 1) * P],
                                    rhs=c_id4[:], start=True, stop=True)
                            ex_sb = wp.tile([P, SUPER * HEADS], f32, tag='exs')
                            nc.vector.tensor_copy(ex_sb[:, :nch * HEADS],
                                                  ps_ex[:, :nch * HEADS])

                            # xj edge-major [e, f]
                            ps_xj = psD.tile([P, SUPER * P], f32, tag='xj')
                            for j in range(nch):
                                nc.tensor.matmul(
                                    ps_xj[:, j * P:(j + 1) * P],
                                    lhsT=xsT[:, j * P:(j + 1) * P],
                                    rhs=c_wlT[:], start=True, stop=True)

                            # msg = [xj * ex_bcast | ex]  -> [128, nch, 132]
                            msg = wp.tile([P, SUPER, 132], f32, tag='msg')
                            xj_v = ps_xj[:, :W].rearrange('p (c f) -> p c f', c=nch)
                            if aff is not None:
                                # general b_l: xj += b_l (broadcast over rows)
                                xj_sb = wp.tile([P, SUPER * P], f32, tag='xjb')
                                blv = c_aff[:, 0:P][:, None, :].to_broadcast(
                                    [P, nch, P])
                                nc.vector.tensor_tensor(
                                    out=xj_sb[:, :W].rearrange(
                                        'p (c f) -> p c f', c=nch),
                                    in0=xj_v, in1=blv, op=OP.add)
                                xj_v = xj_sb[:, :W].rearrange(
                                    'p (c f) -> p c f', c=nch)
                            ex_v = (ex_sb[:, :nch * HEADS]
                                    .rearrange('p (c h) -> p c h', c=nch)
                                    [:, :, :, None].to_broadcast(
                                        [P, nch, HEADS, HEAD_DIM]))
                            nc.vector.tensor_tensor(
                                out=msg[:, :nch, 0:P].rearrange(
                                    'p c (h d) -> p c h d', h=HEADS),
                                in0=xj_v.rearrange(
                                    'p c (h d) -> p c h d', h=HEADS),
                                in1=ex_v, op=OP.mult)
                            nc.vector.tensor_copy(
                                msg[:, :nch, P:P + HEADS],
                                ex_sb[:, :nch * HEADS].rearrange(
                                    'p (c h) -> p c h', c=nch))

                            # indicator I[e, c, n] = (dst_local == n)
                            I = wp.tile([P, SUPER, P], f32, tag='I')
                            iota_v = c_iota[:, None, :].to_broadcast([P, nch, P])
                            dl_v = (dl_t[:, s * SUPER:s * SUPER + nch]
                                    [:, :, None].to_broadcast([P, nch, P]))
                            nc.vector.tensor_tensor(
                                out=I[:, :nch, :], in0=iota_v, in1=dl_v,
                                op=OP.is_equal)

                            # scatter: ps_out[n, :] += I^T @ msg
                            for j in range(nch):
                                first = (s == 0 and j == 0)
                                last = (s == n_super - 1 and j == nch - 1)
                                nc.tensor.matmul(ps_out[:], lhsT=I[:, j, :],
                                                 rhs=msg[:, j, :],
                                                 start=first, stop=last)
                        nc.vector.tensor_copy(
                            ubuf[:, t * 132:(t + 1) * 132], ps_out[:])
                        chunk_base += Ct

                # ---------- phase 3: normalize + silu + residual + LN ----------
                with tc.tile_pool(name='tail', bufs=3) as tp:
                    for t in range(NT):
                        u_sl = ubuf[:, t * 132:t * 132 + P]
                        d_sl = ubuf[:, t * 132 + P:t * 132 + P + HEADS]
                        rv = tp.tile([P, HEADS], f32, tag='rv')
                        nc.vector.tensor_scalar(
                            out=rv[:], in0=d_sl, scalar1=1e-16, scalar2=None,
                            op0=OP.add)
                        rvi = tp.tile([P, HEADS], f32, tag='rvi')
                        nc.vector.reciprocal(rvi[:], rv[:])
                        u = tp.tile([P, P], f32, tag='u')
                        rvi_v = rvi[:, :, None].to_broadcast(
                            [P, HEADS, HEAD_DIM])
                        nc.vector.tensor_tensor(
                            out=u[:].rearrange('p (h d) -> p h d', h=HEADS),
                            in0=u_sl.rearrange('p (h d) -> p h d', h=HEADS),
                            in1=rvi_v, op=OP.mult)
                        if aff is not None:
                            nc.vector.tensor_tensor(
                                out=u[:], in0=u[:], in1=c_aff[:, P:2 * P],
                                op=OP.add)
                        ss = tp.tile([P, P], f32, tag='ss')
                        nc.scalar.activation(ss[:], u[:], AT.Silu)
                        xo = tp.tile([P, P], f32, tag='xo')
                        nc.sync.dma_start(xo[:], x_own[t * P:(t + 1) * P, :])
                        h_sl = hbuf[:, t * P:(t + 1) * P]
                        nc.vector.tensor_tensor(out=h_sl, in0=ss[:], in1=xo[:],
                                                op=OP.add)
                        bs = tp.tile([P, 6], f32, tag='bs')
                        nc.vector.bn_stats(bs[:], h_sl)
                        nc.vector.bn_aggr(stats[:, t * 2:t * 2 + 2], bs[:])

                    veps = tp.tile([P, NT], f32, tag='veps')
                    var_v = stats[:].rearrange('p (t k) -> p t k', k=2)[:, :, 1]
                    nc.vector.tensor_scalar(out=veps[:], in0=var_v,
                                            scalar1=LN_EPS, scalar2=None,
                                            op0=OP.add)
                    vinv = tp.tile([P, NT], f32, tag='vinv')
                    nc.vector.reciprocal(vinv[:], veps[:])
                    rstd = tp.tile([P, NT], f32, tag='rstd')
                    nc.scalar.activation(rstd[:], vinv[:], AT.Sqrt)

                    for t in range(NT):
                        o = tp.tile([P, P], f32, tag='o')
                        nc.vector.tensor_scalar(
                            out=o[:], in0=hbuf[:, t * P:(t + 1) * P],
                            scalar1=stats[:, t * 2:t * 2 + 1],
                            scalar2=rstd[:, t:t + 1],
                            op0=OP.subtract, op1=OP.mult)
                        if aff is not None:
                            nc.vector.tensor_tensor(
                                out=o[:], in0=o[:], in1=c_aff[:, 2 * P:3 * P],
                                op=OP.mult)
                            nc.vector.tensor_tensor(
                                out=o[:], in0=o[:], in1=c_aff[:, 3 * P:4 * P],
                                op=OP.add)
                        nc.sync.dma_start(out_d[t * P:(t + 1) * P, :], o[:])

    nc.compile()
    return nc


def kernel(x, edge_index, edge_attr, w_l, b_l, w_r, b_r, w_e, att,
           conv_bias, ln_gamma, ln_beta):
    from concourse.bass_utils import run_bass_kernel_spmd

    x = np.asarray(x, dtype=np.float32)
    edge_index = np.asarray(edge_index)
    edge_attr = np.asarray(edge_attr, dtype=np.float32)
    w_l = np.asarray(w_l, dtype=np.float32)
    b_l = np.asarray(b_l, dtype=np.float32)
    w_r = np.asarray(w_r, dtype=np.float32)
    b_r = np.asarray(b_r, dtype=np.float32)
    w_e = np.asarray(w_e, dtype=np.float32)
    att = np.asarray(att, dtype=np.float32)
    conv_bias = np.asarray(conv_bias, dtype=np.float32)
    ln_gamma = np.asarray(ln_gamma, dtype=np.float32)
    ln_beta = np.asarray(ln_beta, dtype=np.float32)

    N = x.shape[0]
    E = edge_index.shape[1]
    NPC = (N + N_CORES - 1) // N_CORES          # 6250
    NT = (NPC + P - 1) // P                     # 49
    NPC_PAD = NT * P                            # 6272

    src = edge_index[0].astype(np.int64)
    dst = edge_index[1].astype(np.int64)
    core = np.minimum(dst // NPC, N_CORES - 1)

    trivial_affine = (not b_l.any()) and (not conv_bias.any()) and \
        np.all(ln_gamma == 1.0) and (not ln_beta.any())

    # per (core, tile) edge lists, sorted by dst
    order = np.lexsort((dst,))
    src_s, dst_s, core_s = src[order], dst[order], core[order]
    attr_s = edge_attr[order]
    tile_of = (dst_s - core_s * NPC) // P

    counts = np.zeros((N_CORES, NT), dtype=np.int64)
    np.add.at(counts, (core_s, tile_of), 1)
    C_list = [int(max(1, np.max((counts[:, t] + P - 1) // P)))
              for t in range(NT)]
    TOTAL_CHUNKS = sum(C_list)
    EW = TOTAL_CHUNKS * P

    key = (tuple(C_list), trivial_affine)
    if key in _CACHE:
        nc = _CACHE[key]
    else:
        nc = _build_program(C_list, trivial_affine)
        _CACHE[key] = nc

    # chunk start offsets per tile
    tile_chunk0 = np.zeros(NT, dtype=np.int64)
    acc = 0
    for t in range(NT):
        tile_chunk0[t] = acc
        acc += C_list[t]

    # consts shared by all cores
    w_lT_h = np.ascontiguousarray(w_l.T)
    w_rT_h = np.ascontiguousarray(w_r.T)
    w_eT_h = np.ascontiguousarray(w_e.T)
    att_exp_h = np.zeros((P, HEADS), dtype=np.float32)
    for h in range(HEADS):
        att_exp_h[h * HEAD_DIM:(h + 1) * HEAD_DIM, h] = att[h]
    iota_row_h = np.broadcast_to(
        np.arange(P, dtype=np.float32), (P, P)).copy()
    iota_col_h = np.arange(P, dtype=np.float32)[:, None].copy()
    ones_row_h = np.ones((1, P), dtype=np.float32)
    id4_h = np.eye(HEADS, dtype=np.float32)
    bias_lr_h = (b_l + b_r)[:, None].astype(np.float32).copy()
    aff_h = None
    if not trivial_affine:
        aff_h = np.concatenate([
            np.broadcast_to(b_l, (P, P)),
            np.broadcast_to(conv_bias, (P, P)),
            np.broadcast_to(ln_gamma, (P, P)),
            np.broadcast_to(ln_beta, (P, P))], axis=1).astype(np.float32).copy()

    in_maps = []
    for k in range(N_CORES):
        sel = core_s == k
        ksrc, kdst, ktile = src_s[sel], dst_s[sel], tile_of[sel]
        kattr = attr_s[sel]
        # position of each edge in the padded layout
        # edges already sorted by dst -> grouped by tile, in order
        pos = np.empty(len(ksrc), dtype=np.int64)
        csum = 0
        x_srcT_h = np.zeros((P, EW), dtype=np.float32)
        attrT_h = np.zeros((EDGE_DIM, EW), dtype=np.float32)
        dstrow_h = np.full((1, EW), -1.0, dtype=np.float32)
        dstloc_h = np.full((P, TOTAL_CHUNKS), -1.0, dtype=np.float32)
        for t in range(NT):
            tsel = ktile == t
            n_t = int(tsel.sum())
            base = tile_chunk0[t] * P
            pos[tsel] = base + np.arange(n_t)
            csum += n_t
        x_srcT_h[:, pos] = x[ksrc].T
        attrT_h[:, pos] = kattr.T
        dloc = (kdst - k * NPC - ktile * P).astype(np.float32)
        dstrow_h[0, pos] = dloc
        dstloc_h[pos % P, pos // P] = dloc

        xk = np.zeros((NPC_PAD, P), dtype=np.float32)
        n_own = min(NPC, N - k * NPC)
        xk[:n_own] = x[k * NPC:k * NPC + n_own]
        im = {
            'x_srcT': x_srcT_h, 'attrT': attrT_h, 'dstrow': dstrow_h,
            'dstloc': dstloc_h,
            'x_ownT': np.ascontiguousarray(xk.T), 'x_own': xk,
            'w_lT': w_lT_h, 'w_rT': w_rT_h, 'w_eT': w_eT_h,
            'att_exp': att_exp_h, 'iota_row': iota_row_h,
            'iota_col': iota_col_h, 'ones_row': ones_row_h, 'id4': id4_h,
            'bias_lr': bias_lr_h,
        }
        if aff_h is not None:
            im['aff'] = aff_h
        in_maps.append(im)

    res = run_bass_kernel_spmd(nc, in_maps, list(range(N_CORES)))
    outs = []
    for k in range(N_CORES):
        n_own = min(NPC, N - k * NPC)
        outs.append(res.results[k]['out'][:n_own])
    return np.concatenate(outs, axis=0)
